# revision 81
# baseline (speedup 1.0000x reference)
"""Distributed Bass kernel for nn_AttentionLayer_88545045774526.

Causal attention layer: B=2, N=2048, D=1024, H=16 heads of HD=64.
Sharding: tensor-parallel over heads -- each of the 8 cores owns 2 heads
(both batches, all tokens). The attention output is redistributed
head-sharded -> token-sharded via 4 pipelined AllToAlls (one per 1024
flat tokens, i.e. per 2 q-shards), so the output projection for early
token chunks overlaps attention compute for later shards. Each core
projects 4 chunks of 128 tokens (tokens j*1024 + c*128 for chunk j).

Matmul dtype: bf16 (fp32 streams at 1/4 PE rate); softmax in fp32.
Layout choices (all driven by the PE cost model: matmul time ~ output
free-size, independent of contraction):
 - Scores are computed transposed (S^T[k, q]) per 128-row k-block pair,
   exp'd in one ACT op per pair per head.
 - PV is computed *q-major*: out[q, v] with free-size 65 per k-block
   (vs 512 for the v-major variant), with the softmax denominator in a
   ones-column of V -> lands as a per-partition scalar so normalization
   is reciprocal + tensor_scalar_mul (no partition-broadcast needed).
 - V is computed directly in natural layout (X @ Wv, tokens on
   partitions) -- no PE transposes of V^T; bias is added during the
   PSUM evacuation from a pre-broadcast bias tile.
 - O^T for the projection is produced by [128,128] PE transposes of the
   normalized O. PSUM evacuations all ride DVE (GPSIMD cannot touch PSUM).
 - PSUM accumulation groups never interleave within a bank (hardware
   constraint): PV for each q-subblock runs as one contiguous burst.
 - Dependency-free PE transposes (pe_warm) bridge known stalls so the
   tensor engine's p-state stays at 2.4GHz for the projection tail.
"""

import os
import sys

sys.path.insert(0, "/opt/trn_rl_repo")

import numpy as np
import ml_dtypes

import concourse.bass as bass
import concourse.mybir as mybir
import concourse.tile as tile
from concourse import bacc
from concourse import bass_utils
from concourse.masks import make_identity

BF16 = mybir.dt.bfloat16
F32 = mybir.dt.float32
NPBF16 = ml_dtypes.bfloat16

B, N, D = 2, 2048, 1024
H, HD = 16, 64
NC = 8                 # cores
HPC = 2                # heads per core
NT = B * N             # 4096 flat tokens (batch-major)
TB = 512               # token block (q-shard size)
CH = 128               # output chunk tokens per core per collective
P = 128

_BUILD_CACHE = {}
LAST_RESULT = None     # BassKernelResults of the most recent run (for test.py)


def _build_module(sim_mode=False, amplify=1, no_cc=False, probe="none", debug_out=False):
    """Build + compile the SPMD Bass graph (identical on all 8 cores).

    sim_mode=True replaces the AllToAlls with local DMAs so the
    single-core TimelineSim cost model can run (no collectives there).
    amplify=N repeats the whole compute body N times (timing
    amplification). no_cc=True swaps AllToAlls for local DMAs on the
    multi-core build (timing probe only; output is wrong).
    """
    key = (("nc_sim" if sim_mode else "nc") + (f"_x{amplify}" if amplify > 1 else "")
           + ("_nocc" if no_cc else "") + (f"_{probe}" if probe != "none" else "")
           + ("_dbg" if debug_out else ""))
    if key in _BUILD_CACHE:
        return _BUILD_CACHE[key]

    nc = bacc.Bacc(
        "TRN2",
        target_bir_lowering=False,
        debug=False,
        enable_asserts=False,
        num_devices=1 if sim_mode else NC,
    )

    # ---- I/O ----
    xt = nc.dram_tensor("xt", [D, NT], BF16, kind="ExternalInput")        # X^T, replicated
    wqkv = nc.dram_tensor("wqkv", [D, 3 * P], BF16, kind="ExternalInput")  # [Q|K|V] cols, 2 heads
    bqkv = nc.dram_tensor("bqkv", [3 * P], F32, kind="ExternalInput")
    wp = nc.dram_tensor("wp", [D, D], BF16, kind="ExternalInput")          # full W_proj
    bp = nc.dram_tensor("bp", [D], F32, kind="ExternalInput")
    tri = nc.dram_tensor("tri", [P, P], BF16, kind="ExternalInput")        # tri[p,f]=1 if f>=p
    out = nc.dram_tensor("out", [4 * CH, D], BF16, kind="ExternalOutput")  # 4 chunks of 128 tokens

    # collective bounce buffers (internal DRAM), one pair per chunk
    cc_in = [nc.dram_tensor(f"cc_in{j}", [NC * P, CH], BF16, kind="Internal")
             for j in range(4)]
    cc_out = [nc.dram_tensor(f"cc_out{j}", [NC * P, CH], BF16, kind="Internal")
              for j in range(4)]
    if debug_out:
        dbg_ccin = [nc.dram_tensor(f"dbg_ccin{j}", [NC * P, CH], BF16,
                                   kind="ExternalOutput") for j in range(4)]
        dbg_ccout = [nc.dram_tensor(f"dbg_ccout{j}", [NC * P, CH], BF16,
                                    kind="ExternalOutput") for j in range(4)]
        dbg_qt = nc.dram_tensor("dbg_qt", [P, NT], BF16, kind="ExternalOutput")
        dbg_kt = nc.dram_tensor("dbg_kt", [P, NT], BF16, kind="ExternalOutput")
        dbg_vs = nc.dram_tensor("dbg_vs", [P, 32 * 130], BF16,
                                kind="ExternalOutput")
        dbg_onat = nc.dram_tensor("dbg_onat", [P, 8 * 4 * P], BF16,
                                  kind="ExternalOutput")

    NTB = NT // TB          # 8 token blocks
    DC = D // P             # 8 contraction chunks
    KBB = N // P            # 16 k-blocks per batch
    QB = N // TB            # 4 q-blocks per batch
    QS = TB // P            # 4 q-subblocks per shard

    with tile.TileContext(nc) as tc:
        with (
            tc.tile_pool(name="consts", bufs=1) as consts,
            tc.tile_pool(name="xt_pool", bufs=1) as xt_pool,
            tc.tile_pool(name="pers", bufs=1) as pers,
            tc.tile_pool(name="mm_psum", bufs=2, space="PSUM") as mm_psum,
            tc.tile_pool(name="pss_psum", bufs=2, space="PSUM") as pss_psum,
            tc.tile_pool(name="pso_psum", bufs=1, space="PSUM") as pso_psum,
            tc.tile_pool(name="work", bufs=4) as work,
            tc.tile_pool(name="small", bufs=4) as small,
        ):
            # ---- constants / first-needed weights to SBUF ----
            # order matters: everything here shares one serial DMA pipeline,
            # so load in need-order and with few, large transfers
            wqkv_sb = consts.tile([P, DC, 3 * P], BF16)
            nc.sync.dma_start(
                wqkv_sb, wqkv[:, :].rearrange("(c p) n -> p c n", p=P))
            xt_sb = xt_pool.tile([P, DC, NT], BF16)
            # first quarter-block separately so the first matmuls start early
            nc.sync.dma_start(
                xt_sb[:, :, 0:P * 2],
                xt[:, 0:P * 2].rearrange("(c p) n -> p c n", p=P))
            nc.sync.dma_start(
                xt_sb[:, :, P * 2:TB],
                xt[:, P * 2:TB].rearrange("(c p) n -> p c n", p=P))
            bq_sb = consts.tile([P, 3], F32)
            nc.sync.dma_start(bq_sb, bqkv[:].rearrange("(g p) -> p g", p=P))
            tri_sb = consts.tile([P, P], BF16)
            nc.sync.dma_start(tri_sb, tri[:, :])
            # V-bias broadcast down partitions (tokens) -- needed by the very
            # first V evacuation; DRAM source APs may broadcast.
            bv_bc = consts.tile([P, P], F32)
            nc.sync.dma_start(bv_bc, bqkv[2 * P:3 * P][None, :].to_broadcast((P, P)))
            ident = consts.tile([P, P], BF16)
            make_identity(nc, ident)
            # preload the exp table set so the ~1.3us ACT_TABLE_LOAD hides here
            actwarm = consts.tile([1, 1], F32)
            nc.scalar.activation(actwarm, bq_sb[0:1, 0:1],
                                 mybir.ActivationFunctionType.Exp)

            for tb in range(1, 4):
                nc.sync.dma_start(
                    xt_sb[:, :, tb * TB:(tb + 1) * TB],
                    xt[:, tb * TB:(tb + 1) * TB].rearrange("(c p) n -> p c n", p=P),
                )
            # needed from the first tail chunk (~20us in) -- before batch 1 xt
            bp_bc = consts.tile([P, D], F32)
            nc.sync.dma_start(bp_bc, bp[None, :].to_broadcast((P, D)))
            wp_sb = consts.tile([P, DC, D], BF16)
            nc.sync.dma_start(wp_sb, wp[:, :].rearrange("(c p) n -> p c n", p=P))
            for tb in range(4, NTB):
                nc.sync.dma_start(
                    xt_sb[:, :, tb * TB:(tb + 1) * TB],
                    xt[:, tb * TB:(tb + 1) * TB].rearrange("(c p) n -> p c n", p=P),
                )

            # ---- persistent activations ----
            qt = pers.tile([P, NT], BF16)   # Q^T: head h rows h*64..h*64+64
            kt = pers.tile([P, NT], BF16)   # K^T
            # vstore[:, kbg, h*65 : h*65+64] = V rows (natural layout) for
            # k-block kbg, head h; vstore[:, kbg, h*65+64] = 1.0 (softmax
            # denominator ones-column)
            vstore = pers.tile([P, NT // P, HPC * (HD + 1)], BF16)
            nc.vector.memset(vstore[:, :, HD:HD + 1], 1.0)
            nc.vector.memset(vstore[:, :, 2 * HD + 1:2 * HD + 2], 1.0)

            def evac_engine():
                # PSUM evacuation must stay off GPSIMD (no PSUM access)
                return nc.vector

            def qkv_pieces(tb, rep=0):
                # pieces of the QKV projection for token block tb; thunks so
                # they can interleave into the (ACT-bound) attention stretch
                dst = {0: qt, 1: kt}

                def qk_piece(cg, t0=0, t1=TB):
                    # two half-pieces sharing one PSUM accumulation group
                    # (cross-bank interleave with the woven attention
                    # matmuls is exact on HW -- verified); the finer weave
                    # quantum halves S-issue jitter in the PE stream
                    st = {}

                    NSP = 4

                    def half(hh):
                        def go():
                            w = t1 - t0
                            if hh == 0:
                                st["ps"] = mm_psum.tile(
                                    [P, TB], F32, tag="mm_ps", name="ps")
                            pss = st["ps"]
                            step = DC // NSP
                            for dc in range(step * hh, step * (hh + 1)):
                                nc.tensor.matmul(
                                    pss[:, 0:w],
                                    lhsT=wqkv_sb[:, dc,
                                                 cg * P:(cg + 1) * P],
                                    rhs=xt_sb[:, dc,
                                              tb * TB + t0:tb * TB + t1],
                                    start=(dc == 0),
                                    stop=(dc == DC - 1),
                                )
                            if hh == NSP - 1:
                                evac_engine().tensor_add(
                                    out=dst[cg][:,
                                                tb * TB + t0:tb * TB + t1],
                                    in0=pss[:, 0:w],
                                    in1=bq_sb[:, cg:cg + 1].to_broadcast(
                                        (P, w)),
                                )
                        return go
                    return [half(hh) for hh in range(NSP)]

                def v_piece(kbg):
                    def go():
                        # V directly in natural layout: V[tok,:] = X @ Wv
                        psv = mm_psum.tile([P, P], F32, tag="mm_ps",
                                           name="psv")
                        for dc in range(DC):
                            nc.tensor.matmul(
                                psv,
                                lhsT=xt_sb[:, dc, kbg * P:(kbg + 1) * P],
                                rhs=wqkv_sb[:, dc, 2 * P:3 * P],
                                start=(dc == 0), stop=(dc == DC - 1),
                            )
                        # evacuate + bias into the [64|1|64|1] vstore layout
                        eng = evac_engine()
                        for h in range(HPC):
                            eng.tensor_add(
                                out=vstore[:, kbg,
                                           h * (HD + 1):h * (HD + 1) + HD],
                                in0=psv[:, h * HD:(h + 1) * HD],
                                in1=bv_bc[:, h * HD:(h + 1) * HD],
                            )
                    return go

                # Q first: the next shard's first S matmul reads qt of its
                # own block, while kt of that block is only needed by its
                # last (diagonal) pairs and V only by the PV bursts
                if tb == 0:
                    # half-block pieces so the first compute starts on the
                    # first quarter-block xt DMA
                    hb = TB // 2
                    ps = (qk_piece(0, 0, hb) + qk_piece(1, 0, hb)
                          + qk_piece(0, hb, TB) + qk_piece(1, hb, TB))
                else:
                    ps = qk_piece(0) + qk_piece(1)
                for kbg in range(4 * tb, 4 * (tb + 1)):
                    ps.append(v_piece(kbg))
                return ps

            def attn_steps(b, qb, ccst, rep=0):
                # S^T = (K^T block)^T-contraction @ Q^T, exp, then q-major
                # PV. Returned as per-k-pair step thunks; interleave filler
                # work between them (the steps are ACT-bound).
                qoff = b * N + qb * TB
                nkb = 4 * (qb + 1)          # causal: k blocks 0..4qb+3 (even)
                state = {}

                def s_exp(kp):
                    # S matmuls + exp for k-block pair kp
                    pair = (2 * kp, 2 * kp + 1)
                    css = [max(kb - 4 * qb, 0) * P for kb in pair]
                    # two k-blocks share one 2-bank PSUM tile per head so
                    # a single exp covers both (halves ACT per-op cost)
                    psS = [pss_psum.tile([P, 2 * TB], F32, tag="psS",
                                         name=f"psS{h}")
                           for h in range(HPC)]
                    # h-major: psS[h0] completes after 2 matmuls (not 3),
                    # so exp(h0) fires ~0.4us earlier each pair
                    for h in range(HPC):
                        hp = h * HD
                        for i, kb in enumerate(pair):
                            koff = b * N + kb * P
                            nc.tensor.matmul(
                                psS[h][:, i * TB + css[i]:(i + 1) * TB],
                                lhsT=kt[hp:hp + HD, koff:koff + P],
                                rhs=qt[hp:hp + HD,
                                       qoff + css[i]:qoff + TB],
                                start=True, stop=True,
                            )
                    pts = []
                    for h in range(HPC):
                        pt = work.tile([P, 2 * TB], BF16, tag=f"pt{h}",
                                       name="pt", bufs=10)
                        nc.scalar.activation(
                            pt[:, css[0]:2 * TB],
                            psS[h][:, css[0]:2 * TB],
                            mybir.ActivationFunctionType.Exp,
                            scale=float(HD) ** -0.5,
                        )
                        pts.append(pt)
                    return pts

                def mask(kp, pts):
                    # triangular mask on the diagonal squares of pair kp
                    pair = (2 * kp, 2 * kp + 1)
                    css = [max(kb - 4 * qb, 0) * P for kb in pair]
                    for h in range(HPC):
                        for i, kb in enumerate(pair):
                            if kb - 4 * qb >= 0:
                                nc.vector.tensor_mul(
                                    out=pts[h][:, i * TB + css[i]:
                                               i * TB + css[i] + P],
                                    in0=pts[h][:, i * TB + css[i]:
                                               i * TB + css[i] + P],
                                    in1=tri_sb)

                def pv_qs(qs):
                    # q-major PV for q-subblock qs, over all its k-blocks.
                    # PSUM accumulation groups must not interleave within a
                    # bank, so each (h, qs) group's matmuls are contiguous;
                    # this needs all the shard's exp'd pairs alive (pts
                    # pool bufs >= 8).
                    psO = state["psO"]
                    allpts = state["allpts"]
                    for h in range(HPC):
                        for kb in range(0, 4 * qb + qs + 1):
                            kp, i = kb // 2, kb % 2
                            nc.tensor.matmul(
                                psO[h][:, qs, 0:HD + 1],
                                lhsT=allpts[kp][h][:, i * TB + qs * P:
                                                   i * TB + (qs + 1) * P],
                                rhs=vstore[:, b * KBB + kb,
                                           h * (HD + 1):(h + 1) * (HD + 1)],
                                start=(kb == 0),
                                stop=(kb == 4 * qb + qs),
                            )

                def pair_step(kp):
                    # emit S+exp+mask for pair kp; PV bursts are appended
                    # once all the pairs a q-subblock needs have been exp'd
                    def go():
                        if "psO" not in state:
                            # psO[h][:, qs, 0:64] = O accum; [:, qs, 64] = den
                            state["psO"] = [
                                pso_psum.tile([P, QS, P], F32, tag=f"psO{h}",
                                              name=f"psO{h}")
                                for h in range(HPC)]
                            state["allpts"] = []
                        pts = s_exp(kp)
                        mask(kp, pts)
                        state["allpts"].append(pts)
                        if kp == nkb // 2 - 2:      # k-blocks 4qb, 4qb+1 done
                            pv_qs(0)
                            pv_qs(1)
                        elif kp == nkb // 2 - 1:    # all k-blocks done
                            pv_qs(2)
                            pv_qs(3)
                    return go

                def evac():
                    # normalize (denominator = per-partition scalar per qs),
                    # then transpose to O^T and stage for the collective
                    psO = state["psO"]
                    onat = small.tile([P, QS, P], BF16, tag="onat",
                                      name="onat")
                    for h in range(HPC):
                        rec4 = small.tile([P, QS, 1], F32, tag="rec",
                                          name="rec")
                        nc.vector.reciprocal(rec4, psO[h][:, :, HD:HD + 1])
                        nc.vector.tensor_mul(
                            out=onat[:, :, h * HD:(h + 1) * HD],
                            in0=psO[h][:, :, 0:HD],
                            in1=rec4[:, :, :].to_broadcast((P, QS, HD)))
                    p0 = 4 * (qb % 2)
                    if debug_out:
                        sh = b * QB + qb
                        nc.sync.dma_start(
                            dbg_onat[:, sh * QS * P:(sh + 1) * QS * P],
                            onat[:, :, :].rearrange("p a b -> p (a b)"))
                    for qs in range(QS):
                        pst = mm_psum.tile([P, P], BF16, tag="mm_ps",
                                           name="pst")
                        nc.tensor.transpose(pst, onat[:, qs, :], ident)
                        nc.vector.tensor_copy(
                            out=ccst[:, (p0 + qs) * P:(p0 + qs + 1) * P],
                            in_=pst)
                    # ship this shard's half of the collective input now --
                    # the collective then only waits on the second shard
                    tp = (b * QB + qb) // 2
                    if b == 1 and qb == 3:
                        # final shard: per-slab DMAs overlap the transposes,
                        # shortening the end-of-kernel serial chain
                        for qs in range(QS):
                            nc.scalar.dma_start(
                                cc_in[tp][(p0 + qs) * P:(p0 + qs + 1) * P, :],
                                ccst[:, (p0 + qs) * P:(p0 + qs + 1) * P])
                    else:
                        nc.scalar.dma_start(
                            cc_in[tp][p0 * P:(p0 + 4) * P, :].rearrange(
                                "(p q) t -> q p t", q=P),
                            ccst[:, p0 * P:(p0 + 4) * P].rearrange(
                                "q (p t) -> q p t", p=4))

                return [pair_step(kp) for kp in range(nkb // 2)] + [evac]

            def pe_warm(n):
                # dependency-free transposes keep the PE p-state ramped
                # across a known stall (the ramp model drops to 0.65/1.2GHz
                # after any idle; 3us of continuous work restores 2.4GHz)
                for _ in range(n):
                    psw = mm_psum.tile([P, P], BF16, tag="mm_ps", name="psw")
                    nc.tensor.transpose(psw, ident, ident)

            def collective_pieces(tp, ccst):
                def go():
                    # AllToAll chunk tp: cc_in halves were shipped by the
                    # two shards' evacs (ACT ring)
                    if sim_mode or no_cc:
                        nc.scalar.dma_start(cc_out[tp][:, :], cc_in[tp][:, :])
                    else:
                        nc.gpsimd.collective_compute(
                            "AllToAll",
                            mybir.AluOpType.bypass,
                            replica_groups=[list(range(NC))],
                            ins=[cc_in[tp][:, :]],
                            outs=[cc_out[tp][:, :]],
                        )
                return go

            def tail_pieces(tp):
                # output projection for my 128 tokens of chunk tp
                state = {}

                def half(i):
                    def go():
                        if "otf" not in state:
                            otf = work.tile([P, DC, CH], BF16, tag="otf",
                                            name="otf", bufs=2)
                            nc.scalar.dma_start(
                                otf, cc_out[tp][:, :].rearrange(
                                    "(c p) n -> p c n", p=P))
                            state["otf"] = otf
                            state["ysb"] = work.tile([P, D], BF16,
                                                     tag="ysb", name="ysb",
                                                     bufs=2)
                        otf, ysb = state["otf"], state["ysb"]
                        psy = mm_psum.tile([P, TB], F32, tag="mm_ps",
                                           name="psy")
                        for hc in range(DC):
                            nc.tensor.matmul(
                                psy,
                                lhsT=otf[:, hc, :],
                                rhs=wp_sb[:, hc, i * TB:(i + 1) * TB],
                                start=(hc == 0), stop=(hc == DC - 1),
                            )
                        nc.vector.tensor_add(
                            out=ysb[:, i * TB:(i + 1) * TB], in0=psy,
                            in1=bp_bc[:, i * TB:(i + 1) * TB],
                        )
                        # out DMAs per half on the ACT ring (off the busy
                        # SP ring; ACT is idle during the tail)
                        nc.scalar.dma_start(
                            out[tp * CH:(tp + 1) * CH, i * TB:(i + 1) * TB],
                            ysb[:, i * TB:(i + 1) * TB])
                    return go
                return [half(0), half(1)]

            # software pipeline: while the (ACT-bound) attention of shard s
            # runs, weave in (a) the QKV projection for token block s+1 and
            # (b) the output projection for the chunk whose AllToAll
            # completed during the previous attention stretch. Engine
            # streams execute in program order, so emission order here IS
            # the per-engine schedule.
            for rep in range(amplify):
                do_qkv = probe != "no_qkv" or rep == 0
                do_attn = probe != "no_attn" or rep == 0
                do_tail = probe != "no_tail" or rep == 0
                if rep == 0:
                    pe_warm(50)
                for piece in (qkv_pieces(0, rep) if do_qkv else []):
                    piece()
                ccst = None
                for s in range(NTB):
                    tp = s // 2
                    if s % 2 == 0:
                        ccst = work.tile([P, NC * P], BF16, tag="ccst",
                                         name="ccst")
                    pending = []
                    if do_qkv and s + 1 < NTB:
                        pending += qkv_pieces(s + 1, rep)
                    if do_tail and s % 2 == 1 and tp > 0:
                        pending += tail_pieces(tp - 1)
                    steps = (attn_steps(s // 4, s % 4, ccst, rep)
                             if do_attn else [])
                    nst = max(len(steps), 1)
                    npend = len(pending)
                    done = 0
                    for i, st in enumerate(steps):
                        st()
                        target = min(npend, (npend * (i + 2)) // nst)
                        while done < target:
                            pending.pop(0)()
                            done += 1
                    while pending:
                        pending.pop(0)()
                    if do_tail and s % 2 == 1:
                        collective_pieces(tp, ccst)()
                if do_tail:
                    pe_warm(74)
                    for piece in tail_pieces(3):
                        piece()
                if debug_out:
                    nc.sync.dma_start(dbg_qt[:, :], qt)
                    nc.sync.dma_start(dbg_kt[:, :], kt)
                    nc.sync.dma_start(
                        dbg_vs[:, :],
                        vstore[:, :, :].rearrange("p a b -> p (a b)"))
                    for j in range(4):
                        nc.sync.dma_start(dbg_ccin[j][:, :], cc_in[j][:, :])
                        nc.sync.dma_start(dbg_ccout[j][:, :], cc_out[j][:, :])

    nc.compile()
    _BUILD_CACHE[key] = nc
    return nc


def _make_in_maps(hidden_states, W_attn, b_attn, W_proj, b_proj):
    x = np.asarray(hidden_states, dtype=np.float32).reshape(NT, D)
    xt = np.ascontiguousarray(x.T).astype(NPBF16)
    wp = np.asarray(W_proj, dtype=np.float32).astype(NPBF16)
    bp = np.asarray(b_proj, dtype=np.float32)
    W = np.asarray(W_attn, dtype=np.float32)
    bias = np.asarray(b_attn, dtype=np.float32)
    # tri[p, f] = 1 where f >= p   (keep q >= k within the diagonal square)
    tri = (np.arange(P)[None, :] >= np.arange(P)[:, None]).astype(NPBF16)

    in_maps = []
    for c in range(NC):
        h0 = HPC * c
        cols = slice(h0 * HD, h0 * HD + HPC * HD)
        wqkv = np.concatenate(
            [W[:, cols], W[:, D:][:, cols], W[:, 2 * D:][:, cols]], axis=1)
        bq = np.concatenate(
            [bias[cols], bias[D:][cols], bias[2 * D:][cols]])
        in_maps.append({
            "xt": xt,
            "wqkv": np.ascontiguousarray(wqkv).astype(NPBF16),
            "bqkv": np.ascontiguousarray(bq),
            "wp": wp,
            "bp": bp,
            "tri": tri,
        })
    return in_maps


def kernel(**inputs):
    global LAST_RESULT
    nc = _build_module()
    in_maps = _make_in_maps(**inputs)
    trace = os.environ.get("KERNEL_TRACE", "0") == "1"
    res = bass_utils.run_bass_kernel_spmd(
        nc, in_maps, core_ids=list(range(NC)), trace=trace)
    LAST_RESULT = res
    y = np.empty((NT, D), dtype=np.float32)
    for c in range(NC):
        yc = np.asarray(res.results[c]["out"], dtype=np.float32)  # [512, 1024]
        for j in range(4):
            y[j * 1024 + c * CH: j * 1024 + (c + 1) * CH, :] = \
                yc[j * CH:(j + 1) * CH, :]
    return y.reshape(B, N, D)


if __name__ == "__main__":
    # smoke test with random inputs
    rng = np.random.default_rng(0)
    inputs = {
        "hidden_states": rng.standard_normal((B, N, D), dtype=np.float32),
        "W_attn": (rng.standard_normal((D, 3 * D), dtype=np.float32) * D ** -0.5),
        "b_attn": rng.standard_normal((3 * D,), dtype=np.float32) * 0.02,
        "W_proj": (rng.standard_normal((D, D), dtype=np.float32) * D ** -0.5),
        "b_proj": rng.standard_normal((D,), dtype=np.float32) * 0.02,
    }
    y = kernel(**inputs)
    print("output", y.shape, y.dtype, float(np.abs(y).mean()))
